# revision 3
# baseline (speedup 1.0000x reference)
"""Trainium2 Bass kernel for the BaseRenderer (NeRF-style volume rendering)
problem: per-ray depth-sort + transmittance compositing + weighted reductions.

Self-contained: builds, compiles and runs a Bass/Tile SPMD kernel on 8
NeuronCores, data-parallel over rays (axis 0).

Algorithm per core (rays-on-partitions, 8 rays packed per partition row):
  1. Key build: P = ((z*2^21 + 2^16) << 7) | sample_idx  (int32). z sits on a
     2^-21 grid, so the key is exact and embeds the stable-argsort tiebreak;
     the bias keeps the bitcast-fp32 view in the normal range so native fp32
     min/max sorts the integer order exactly.
  2. Keys-only bitonic sort (28 stages) on the bitcast-fp32 view.
  3. Unpack sorted z + permutation; rank (inverse perm) via gpsimd
     local_scatter (int16).
  4. sigma -> sorted order via two 16-bit-plane local_scatters.
  5. dists / tau / exp / exclusive-cumprod (tensor_tensor_scan with per-ray
     reset) / w.
  6. w -> original order (two more 16-bit-plane local_scatters); fused
     tensor_tensor_reduce dot products for depth/color/instance.
"""

import numpy as np
from contextlib import ExitStack

N_CORES = 8
N_RAYS = 65536
S = 128
K_INST = 3

_cache = {}


def _build(n_rays_core: int, segs: int = 8):
    import concourse.bacc as bacc
    import concourse.mybir as mybir
    from concourse.tile import TileContext

    FP32 = mybir.dt.float32
    I32 = mybir.dt.int32
    I16 = mybir.dt.int16
    U16 = mybir.dt.uint16
    Alu = mybir.AluOpType
    Act = mybir.ActivationFunctionType

    INF_DIST = 1e10
    EPS = 1e-10
    ZSCALE = float(2 ** 21)

    P = 128
    block_rays = P * segs
    F = segs * S
    n_blocks = n_rays_core // block_rays
    assert n_rays_core % block_rays == 0

    nc = bacc.Bacc("TRN2", target_bir_lowering=False, debug=False,
                   enable_asserts=False)

    z_d = nc.dram_tensor("z", [n_rays_core, S], FP32, kind="ExternalInput")
    sg_d = nc.dram_tensor("sigma", [n_rays_core, S], FP32, kind="ExternalInput")
    rgb_d = nc.dram_tensor("rgb", [n_rays_core, S * 3], FP32, kind="ExternalInput")
    ins_d = nc.dram_tensor("inst", [n_rays_core, S * 3], FP32, kind="ExternalInput")
    bg_d = nc.dram_tensor("bg", [n_rays_core, 3], FP32, kind="ExternalInput")

    col_d = nc.dram_tensor("color", [n_rays_core, 3], FP32, kind="ExternalOutput")
    dep_d = nc.dram_tensor("depth", [n_rays_core, 1], FP32, kind="ExternalOutput")
    io_d = nc.dram_tensor("instance", [n_rays_core, 3], FP32, kind="ExternalOutput")
    w_d = nc.dram_tensor("w", [n_rays_core, S], FP32, kind="ExternalOutput")

    zv = z_d.ap().rearrange("(b p s) k -> b p (s k)", b=n_blocks, p=P, s=segs)
    sgv = sg_d.ap().rearrange("(b p s) k -> b p (s k)", b=n_blocks, p=P, s=segs)
    rgbv = rgb_d.ap().rearrange("(b p s) k -> b p (s k)", b=n_blocks, p=P, s=segs)
    insv = ins_d.ap().rearrange("(b p s) k -> b p (s k)", b=n_blocks, p=P, s=segs)
    bgv = bg_d.ap().rearrange("(b p s) k -> b p (s k)", b=n_blocks, p=P, s=segs)
    colv = col_d.ap().rearrange("(b p s) k -> b p (s k)", b=n_blocks, p=P, s=segs)
    depv = dep_d.ap().rearrange("(b p s) k -> b p (s k)", b=n_blocks, p=P, s=segs)
    iov = io_d.ap().rearrange("(b p s) k -> b p (s k)", b=n_blocks, p=P, s=segs)
    wv = w_d.ap().rearrange("(b p s) k -> b p (s k)", b=n_blocks, p=P, s=segs)

    stages = []
    kk = 2
    while kk <= S:
        j = kk // 2
        while j >= 1:
            stages.append((kk, j))
            j //= 2
        kk *= 2

    with ExitStack() as ctx:
        tc = ctx.enter_context(TileContext(nc))

        const = ctx.enter_context(tc.tile_pool(name="const", bufs=1))
        iota_s = const.tile([P, F], I32, tag="iota_s")
        nc.gpsimd.iota(iota_s[:], pattern=[[0, segs], [1, S]], base=0,
                       channel_multiplier=0)
        iota_g16 = const.tile([P, F], I16, tag="iota_g16")
        nc.gpsimd.iota(iota_g16[:], pattern=[[S, segs], [1, S]], base=0,
                       channel_multiplier=0)
        segbase = const.tile([P, F], I32, tag="segbase")
        nc.gpsimd.iota(segbase[:], pattern=[[S, segs], [0, S]], base=0,
                       channel_multiplier=0)
        reset = const.tile([P, F], FP32, tag="reset")
        nc.vector.memset(reset[:], 0.0)
        nc.vector.memset(
            reset[:].rearrange("p (s k) -> p s k", s=segs)[:, :, 0:1], 1.0)

        main = ctx.enter_context(tc.tile_pool(name="main", bufs=2))
        scr = ctx.enter_context(tc.tile_pool(name="scr", bufs=1))

        for b in range(n_blocks):
            zt = main.tile([P, F], FP32, tag="zt")
            nc.sync.dma_start(zt[:], zv[b])
            sgt = main.tile([P, F], FP32, tag="sgt")
            nc.sync.dma_start(sgt[:], sgv[b])

            kf = scr.tile([P, F], FP32, tag="kf")
            nc.vector.tensor_scalar(kf[:], zt[:], ZSCALE, 65536.0,
                                    Alu.mult, Alu.add)
            ka = scr.tile([P, F], I32, tag="ka")
            nc.vector.tensor_copy(ka[:], kf[:])
            nc.vector.tensor_scalar(ka[:], ka[:], 7, None,
                                    Alu.logical_shift_left)
            nc.vector.tensor_tensor(ka[:], ka[:], iota_s[:], Alu.bitwise_or)

            kb = scr.tile([P, F], I32, tag="kb")
            cur, nxt = ka, kb
            for (kkv, j) in stages:
                width = min(2 * kkv, S)

                def pair_view(t, base):
                    v = t[:].bitcast(FP32).rearrange(
                        "p (s m n) -> p s m n", s=segs, n=width)
                    v = v[:, :, :, base:base + kkv]
                    return v.rearrange("p s m (e f) -> p s m e f", f=2 * j)

                for desc in (0, 1):
                    if desc and 2 * kkv > S:
                        continue
                    base = kkv if desc else 0
                    rv = pair_view(cur, base)
                    wvw = pair_view(nxt, base)
                    lo_r, hi_r = rv[:, :, :, :, 0:j], rv[:, :, :, :, j:2 * j]
                    lo_w, hi_w = wvw[:, :, :, :, 0:j], wvw[:, :, :, :, j:2 * j]
                    if desc:
                        nc.vector.tensor_tensor(lo_w, lo_r, hi_r, Alu.max)
                        nc.vector.tensor_tensor(hi_w, lo_r, hi_r, Alu.min)
                    else:
                        nc.vector.tensor_tensor(lo_w, lo_r, hi_r, Alu.min)
                        nc.vector.tensor_tensor(hi_w, lo_r, hi_r, Alu.max)
                cur, nxt = nxt, cur

            ks = cur

            perm32 = scr.tile([P, F], I32, tag="perm32")
            nc.vector.tensor_scalar(perm32[:], ks[:], 127, None,
                                    Alu.bitwise_and)
            nc.vector.tensor_tensor(perm32[:], perm32[:], segbase[:],
                                    Alu.bitwise_or)
            permg = scr.tile([P, F], I16, tag="permg")
            nc.vector.tensor_copy(permg[:], perm32[:])
            zsrt = main.tile([P, F], FP32, tag="zsrt")
            ki = nxt
            nc.vector.tensor_scalar(ki[:], ks[:], 7, None,
                                    Alu.logical_shift_right)
            nc.vector.tensor_copy(zsrt[:], ki[:])
            nc.vector.tensor_scalar(zsrt[:], zsrt[:], 65536.0, 1.0 / ZSCALE,
                                    Alu.subtract, Alu.mult)

            rankg = scr.tile([P, F], I16, tag="rankg")
            nc.gpsimd.local_scatter(rankg[:], iota_g16[:], permg[:],
                                    channels=P, num_elems=F, num_idxs=F)

            sgs = main.tile([P, F], FP32, tag="sgs")
            h16 = scr.tile([P, F], U16, tag="h16")
            l16 = scr.tile([P, F], U16, tag="l16")
            hs16 = scr.tile([P, F], U16, tag="hs16")
            ls16 = scr.tile([P, F], U16, tag="ls16")
            sg_u = sgt[:].bitcast(U16).rearrange("p (f two) -> p f two", two=2)
            nc.vector.tensor_copy(l16[:], sg_u[:, :, 0:1].squeeze(2))
            nc.vector.tensor_copy(h16[:], sg_u[:, :, 1:2].squeeze(2))
            nc.gpsimd.local_scatter(hs16[:], h16[:], rankg[:],
                                    channels=P, num_elems=F, num_idxs=F)
            nc.gpsimd.local_scatter(ls16[:], l16[:], rankg[:],
                                    channels=P, num_elems=F, num_idxs=F)
            sgs_u = sgs[:].bitcast(U16).rearrange("p (f two) -> p f two", two=2)
            nc.vector.tensor_copy(sgs_u[:, :, 0:1].squeeze(2), ls16[:])
            nc.vector.tensor_copy(sgs_u[:, :, 1:2].squeeze(2), hs16[:])

            d = scr.tile([P, F], FP32, tag="d")
            nc.vector.tensor_tensor(d[:, 0:F - 1], zsrt[:, 1:F],
                                    zsrt[:, 0:F - 1], Alu.subtract)
            d_seg = d[:].rearrange("p (s k) -> p s k", s=segs)
            nc.vector.memset(d_seg[:, :, S - 1:S], INF_DIST)

            tau = scr.tile([P, F], FP32, tag="tau")
            nc.vector.tensor_scalar(sgs[:], sgs[:], 0.0, None, Alu.max)
            nc.vector.tensor_tensor(tau[:], sgs[:], d[:], Alu.mult)
            f = scr.tile([P, F], FP32, tag="f")
            nc.scalar.activation(f[:], tau[:], Act.Exp, bias=0.0, scale=-1.0)
            g = scr.tile([P, F], FP32, tag="g")
            nc.vector.tensor_scalar(g[:], f[:], EPS, None, Alu.add)
            alpha = scr.tile([P, F], FP32, tag="alpha")
            nc.scalar.activation(alpha[:], f[:], Act.Copy, bias=1.0, scale=-1.0)

            gsh = scr.tile([P, F], FP32, tag="gsh")
            nc.vector.tensor_copy(gsh[:, 1:F], g[:, 0:F - 1])
            gsh_seg = gsh[:].rearrange("p (s k) -> p s k", s=segs)
            nc.vector.memset(gsh_seg[:, :, 0:1], 0.0)
            trans = main.tile([P, F], FP32, tag="trans")
            nc.vector.tensor_tensor_scan(trans[:], gsh[:], reset[:], 1.0,
                                         Alu.mult, Alu.max)

            w = main.tile([P, F], FP32, tag="w")
            nc.vector.tensor_tensor(w[:], alpha[:], trans[:], Alu.mult)
            nc.sync.dma_start(wv[b], w[:])

            t_seg = trans[:].rearrange("p (s k) -> p s k", s=segs)
            g_seg = g[:].rearrange("p (s k) -> p s k", s=segs)
            nohit = scr.tile([P, segs], FP32, tag="nohit")
            nc.vector.tensor_tensor(
                nohit[:].unsqueeze(2),
                t_seg[:, :, S - 1:S], g_seg[:, :, S - 1:S], Alu.mult)

            depth_t = scr.tile([P, segs], FP32, tag="depth_t")
            prod = scr.tile([P, F], FP32, tag="prod")
            nc.vector.tensor_tensor(prod[:], w[:], zsrt[:], Alu.mult)
            nc.vector.tensor_reduce(
                depth_t[:].unsqueeze(2),
                prod[:].rearrange("p (s k) -> p s k", s=segs),
                mybir.AxisListType.X, Alu.add)
            nc.sync.dma_start(depv[b], depth_t[:])

            worig = main.tile([P, F], FP32, tag="worig")
            w_u = w[:].bitcast(U16).rearrange("p (f two) -> p f two", two=2)
            nc.vector.tensor_copy(l16[:], w_u[:, :, 0:1].squeeze(2))
            nc.vector.tensor_copy(h16[:], w_u[:, :, 1:2].squeeze(2))
            nc.gpsimd.local_scatter(hs16[:], h16[:], permg[:],
                                    channels=P, num_elems=F, num_idxs=F)
            nc.gpsimd.local_scatter(ls16[:], l16[:], permg[:],
                                    channels=P, num_elems=F, num_idxs=F)
            wo_u = worig[:].bitcast(U16).rearrange("p (f two) -> p f two", two=2)
            nc.vector.tensor_copy(wo_u[:, :, 0:1].squeeze(2), ls16[:])
            nc.vector.tensor_copy(wo_u[:, :, 1:2].squeeze(2), hs16[:])

            rgbt = main.tile([P, F * 3], FP32, tag="rgbt")
            nc.sync.dma_start(rgbt[:], rgbv[b])
            inst = main.tile([P, F * 3], FP32, tag="inst")
            nc.sync.dma_start(inst[:], insv[b])
            bgt = scr.tile([P, segs * 3], FP32, tag="bgt")
            nc.sync.dma_start(bgt[:], bgv[b])

            color_t = scr.tile([P, segs * 3], FP32, tag="color_t")
            inst_t = scr.tile([P, segs * 3], FP32, tag="inst_t")
            rgb_c = rgbt[:].rearrange("p (f c) -> p f c", c=3)
            ins_c = inst[:].rearrange("p (f c) -> p f c", c=3)
            col_c = color_t[:].rearrange("p (s c) -> p s c", s=segs)
            ino_c = inst_t[:].rearrange("p (s c) -> p s c", s=segs)
            for c in range(3):
                nc.vector.tensor_tensor(prod[:], worig[:], rgb_c[:, :, c],
                                        Alu.mult)
                nc.vector.tensor_reduce(
                    col_c[:, :, c].unsqueeze(2),
                    prod[:].rearrange("p (s k) -> p s k", s=segs),
                    mybir.AxisListType.X, Alu.add)
                nc.vector.tensor_tensor(prod[:], worig[:], ins_c[:, :, c],
                                        Alu.mult)
                nc.vector.tensor_reduce(
                    ino_c[:, :, c].unsqueeze(2),
                    prod[:].rearrange("p (s k) -> p s k", s=segs),
                    mybir.AxisListType.X, Alu.add)
            bgw = scr.tile([P, segs * 3], FP32, tag="bgw")
            bg_seg = bgt[:].rearrange("p (s c) -> p s c", s=segs)
            bgw_seg = bgw[:].rearrange("p (s c) -> p s c", s=segs)
            for s in range(segs):
                nc.vector.tensor_scalar(bgw_seg[:, s], bg_seg[:, s],
                                        nohit[:, s:s + 1], None, Alu.mult)
            nc.vector.tensor_tensor(color_t[:], color_t[:], bgw[:], Alu.add)
            nc.sync.dma_start(colv[b], color_t[:])
            nc.sync.dma_start(iov[b], inst_t[:])

    nc.compile()
    return nc


def _get_nc():
    key = (N_RAYS // N_CORES,)
    if key not in _cache:
        _cache[key] = _build(N_RAYS // N_CORES)
    return _cache[key]


def _shard(inputs):
    z = np.ascontiguousarray(np.asarray(inputs["z_vals"], dtype=np.float32))
    sg = np.ascontiguousarray(np.asarray(inputs["sigma_vals"], dtype=np.float32))
    rgb = np.asarray(inputs["rgb_vals"], dtype=np.float32)
    ins = np.asarray(inputs["instance_vals"], dtype=np.float32)
    bg = np.ascontiguousarray(np.asarray(inputs["bg_color"], dtype=np.float32))
    n, s = z.shape
    per = n // N_CORES
    rgb2 = np.ascontiguousarray(rgb.reshape(n, s * 3))
    ins2 = np.ascontiguousarray(ins.reshape(n, s * 3))
    maps = []
    for c in range(N_CORES):
        sl = slice(c * per, (c + 1) * per)
        maps.append({"z": z[sl], "sigma": sg[sl], "rgb": rgb2[sl],
                     "inst": ins2[sl], "bg": bg[sl]})
    return maps


def run_sharded(inputs, trace=False):
    from concourse import bass_utils
    nc = _get_nc()
    maps = _shard(inputs)
    res = bass_utils.run_bass_kernel_spmd(
        nc, maps, core_ids=list(range(N_CORES)), trace=trace)
    color = np.concatenate([r["color"] for r in res.results], 0)
    depth = np.concatenate([r["depth"] for r in res.results], 0)[:, 0]
    instance = np.concatenate([r["instance"] for r in res.results], 0)
    w = np.concatenate([r["w"] for r in res.results], 0)
    return (color, depth, instance, w), res


def kernel(**inputs):
    outs, _ = run_sharded(inputs, trace=False)
    return outs


# revision 9
# speedup vs baseline: 1.0572x; 1.0572x over previous
"""Trainium2 Bass kernel for the BaseRenderer (NeRF-style volume rendering)
problem: per-ray depth-sort + transmittance compositing + weighted reductions.

Self-contained: builds, compiles and runs a Bass/Tile SPMD kernel on 8
NeuronCores, data-parallel over rays (axis 0).

Algorithm per core (rays-on-partitions, 8 rays packed per partition row):
  1. Key build: P = ((z*2^21 + 2^16) << 7) | sample_idx  (int32). z sits on a
     2^-21 grid, so the key is exact and embeds the stable-argsort tiebreak;
     the bias keeps the bitcast-fp32 view in the normal range so native fp32
     min/max sorts the integer order exactly.
  2. Keys-only bitonic sort (28 stages) on the bitcast-fp32 view.
  3. Unpack sorted z + permutation; rank (inverse perm) via gpsimd
     local_scatter (int16).
  4. sigma -> sorted order via two 16-bit-plane local_scatters.
  5. dists / tau / exp / exclusive-cumprod (tensor_tensor_scan with per-ray
     reset) / w.
  6. w -> original order (two more 16-bit-plane local_scatters); fused
     tensor_tensor_reduce dot products for depth/color/instance.
"""

import numpy as np
from contextlib import ExitStack

N_CORES = 8
N_RAYS = 65536
S = 128
K_INST = 3

_cache = {}


def _build(n_rays_core: int, segs: int = 8, gp_sort_blocks: int = 2):
    import concourse.bacc as bacc
    import concourse.mybir as mybir
    from concourse.tile import TileContext

    FP32 = mybir.dt.float32
    I32 = mybir.dt.int32
    I16 = mybir.dt.int16
    U16 = mybir.dt.uint16
    Alu = mybir.AluOpType
    Act = mybir.ActivationFunctionType

    INF_DIST = 1e10
    EPS = 1e-10
    ZSCALE = float(2 ** 21)

    P = 128
    block_rays = P * segs
    F = segs * S
    n_blocks = n_rays_core // block_rays
    assert n_rays_core % block_rays == 0

    nc = bacc.Bacc("TRN2", target_bir_lowering=False, debug=False,
                   enable_asserts=False)

    z_d = nc.dram_tensor("z", [n_rays_core, S], FP32, kind="ExternalInput")
    sg_d = nc.dram_tensor("sigma", [n_rays_core, S], FP32, kind="ExternalInput")
    rgb_d = nc.dram_tensor("rgb", [n_rays_core, S * 3], FP32, kind="ExternalInput")
    ins_d = nc.dram_tensor("inst", [n_rays_core, S * 3], FP32, kind="ExternalInput")
    bg_d = nc.dram_tensor("bg", [n_rays_core, 3], FP32, kind="ExternalInput")

    col_d = nc.dram_tensor("color", [n_rays_core, 3], FP32, kind="ExternalOutput")
    dep_d = nc.dram_tensor("depth", [n_rays_core, 1], FP32, kind="ExternalOutput")
    io_d = nc.dram_tensor("instance", [n_rays_core, 3], FP32, kind="ExternalOutput")
    w_d = nc.dram_tensor("w", [n_rays_core, S], FP32, kind="ExternalOutput")

    zv = z_d.ap().rearrange("(b p s) k -> b p (s k)", b=n_blocks, p=P, s=segs)
    sgv = sg_d.ap().rearrange("(b p s) k -> b p (s k)", b=n_blocks, p=P, s=segs)
    rgbv = rgb_d.ap().rearrange("(b p s) k -> b p (s k)", b=n_blocks, p=P, s=segs)
    insv = ins_d.ap().rearrange("(b p s) k -> b p (s k)", b=n_blocks, p=P, s=segs)
    bgv = bg_d.ap().rearrange("(b p s) k -> b p (s k)", b=n_blocks, p=P, s=segs)
    colv = col_d.ap().rearrange("(b p s) k -> b p (s k)", b=n_blocks, p=P, s=segs)
    depv = dep_d.ap().rearrange("(b p s) k -> b p (s k)", b=n_blocks, p=P, s=segs)
    iov = io_d.ap().rearrange("(b p s) k -> b p (s k)", b=n_blocks, p=P, s=segs)
    wv = w_d.ap().rearrange("(b p s) k -> b p (s k)", b=n_blocks, p=P, s=segs)

    stages = []
    kk = 2
    while kk <= S:
        j = kk // 2
        while j >= 1:
            stages.append((kk, j))
            j //= 2
        kk *= 2

    with ExitStack() as ctx:
        tc = ctx.enter_context(TileContext(nc))

        const = ctx.enter_context(tc.tile_pool(name="const", bufs=1))
        iota_s = const.tile([P, F], I32, tag="iota_s")
        nc.gpsimd.iota(iota_s[:], pattern=[[0, segs], [1, S]], base=0,
                       channel_multiplier=0)
        iota_g16 = const.tile([P, F], I16, tag="iota_g16")
        nc.gpsimd.iota(iota_g16[:], pattern=[[S, segs], [1, S]], base=0,
                       channel_multiplier=0)
        segbase = const.tile([P, F], I32, tag="segbase")
        nc.gpsimd.iota(segbase[:], pattern=[[S, segs], [0, S]], base=0,
                       channel_multiplier=0)
        reset = const.tile([P, F], FP32, tag="reset")
        nc.vector.memset(reset[:], 0.0)
        nc.vector.memset(
            reset[:].rearrange("p (s k) -> p s k", s=segs)[:, :, 0:1], 1.0)

        main = ctx.enter_context(tc.tile_pool(name="main", bufs=2))
        scr = ctx.enter_context(tc.tile_pool(name="scr", bufs=1))

        for b in range(n_blocks):
            # engine for this block's sort compare-exchanges
            sort_eng = (nc.gpsimd if b >= n_blocks - gp_sort_blocks
                        else nc.vector)
            zt = main.tile([P, F], FP32, tag="zt")
            nc.sync.dma_start(zt[:], zv[b])
            sgt = main.tile([P, F], FP32, tag="sgt")
            nc.sync.dma_start(sgt[:], sgv[b])

            kf = scr.tile([P, F], FP32, tag="kf")
            nc.vector.tensor_scalar(kf[:], zt[:], ZSCALE, 65536.0,
                                    Alu.mult, Alu.add)
            ka = scr.tile([P, F], I32, tag="ka")
            nc.gpsimd.tensor_copy(ka[:], kf[:])
            nc.vector.tensor_scalar(ka[:], ka[:], 7, None,
                                    Alu.logical_shift_left)
            nc.vector.tensor_tensor(ka[:], ka[:], iota_s[:], Alu.bitwise_or)

            kb = scr.tile([P, F], I32, tag="kb")
            cur, nxt = ka, kb
            for (kkv, j) in stages:
                cf = cur[:].bitcast(FP32)
                nf = nxt[:].bitcast(FP32)
                if j == kkv // 2 and kkv > 1:
                    # flip stage: pairs (t, kkv-1-t) within each kkv block
                    half = kkv // 2
                    a_r = cf.rearrange("p (c t) -> p c t", t=kkv)[:, :, 0:half]
                    b_r = cf.rearrange("p (c t) -> p c t", t=kkv)[
                        :, :, kkv - 1:half - 1:-1]
                    a_w = nf.rearrange("p (c t) -> p c t", t=kkv)[:, :, 0:half]
                    b_w = nf.rearrange("p (c t) -> p c t", t=kkv)[
                        :, :, kkv - 1:half - 1:-1]
                else:
                    # normal stage: pairs (i, i+j), all ascending
                    v = cur[:].bitcast(FP32).rearrange(
                        "p (q f) -> p q f", f=2 * j)
                    wvx = nxt[:].bitcast(FP32).rearrange(
                        "p (q f) -> p q f", f=2 * j)
                    a_r, b_r = v[:, :, 0:j], v[:, :, j:2 * j]
                    a_w, b_w = wvx[:, :, 0:j], wvx[:, :, j:2 * j]
                nc.vector.tensor_tensor(a_w, a_r, b_r, Alu.min)
                nc.vector.tensor_tensor(b_w, a_r, b_r, Alu.max)
                cur, nxt = nxt, cur

            ks = cur

            perm32 = scr.tile([P, F], I32, tag="perm32")
            nc.vector.tensor_scalar(perm32[:], ks[:], 127, None,
                                    Alu.bitwise_and)
            nc.vector.tensor_tensor(perm32[:], perm32[:], segbase[:],
                                    Alu.bitwise_or)
            permg = scr.tile([P, F], I16, tag="permg")
            nc.gpsimd.tensor_copy(permg[:], perm32[:])
            zsrt = main.tile([P, F], FP32, tag="zsrt")
            ki = nxt
            nc.vector.tensor_scalar(ki[:], ks[:], 7, None,
                                    Alu.logical_shift_right)
            nc.gpsimd.tensor_copy(zsrt[:], ki[:])
            nc.vector.tensor_scalar(zsrt[:], zsrt[:], 65536.0, 1.0 / ZSCALE,
                                    Alu.subtract, Alu.mult)

            rankg = scr.tile([P, F], I16, tag="rankg")
            nc.gpsimd.local_scatter(rankg[:], iota_g16[:], permg[:],
                                    channels=P, num_elems=F, num_idxs=F)

            sgs = main.tile([P, F], FP32, tag="sgs")
            h16 = scr.tile([P, F], U16, tag="h16")
            l16 = scr.tile([P, F], U16, tag="l16")
            hs16 = scr.tile([P, F], U16, tag="hs16")
            ls16 = scr.tile([P, F], U16, tag="ls16")
            sg_u = sgt[:].bitcast(U16).rearrange("p (f two) -> p f two", two=2)
            nc.scalar.copy(l16[:], sg_u[:, :, 0:1].squeeze(2))
            nc.scalar.copy(h16[:], sg_u[:, :, 1:2].squeeze(2))
            nc.gpsimd.local_scatter(hs16[:], h16[:], rankg[:],
                                    channels=P, num_elems=F, num_idxs=F)
            nc.gpsimd.local_scatter(ls16[:], l16[:], rankg[:],
                                    channels=P, num_elems=F, num_idxs=F)
            sgs_u = sgs[:].bitcast(U16).rearrange("p (f two) -> p f two", two=2)
            nc.scalar.copy(sgs_u[:, :, 0:1].squeeze(2), ls16[:])
            nc.scalar.copy(sgs_u[:, :, 1:2].squeeze(2), hs16[:])

            d = scr.tile([P, F], FP32, tag="d")
            nc.vector.tensor_tensor(d[:, 0:F - 1], zsrt[:, 1:F],
                                    zsrt[:, 0:F - 1], Alu.subtract)
            d_seg = d[:].rearrange("p (s k) -> p s k", s=segs)
            nc.gpsimd.memset(d_seg[:, :, S - 1:S], INF_DIST)

            tau = scr.tile([P, F], FP32, tag="tau")
            nc.scalar.activation(sgs[:], sgs[:], Act.Relu)
            nc.vector.tensor_tensor(tau[:], sgs[:], d[:], Alu.mult)
            f = scr.tile([P, F], FP32, tag="f")
            nc.scalar.activation(f[:], tau[:], Act.Exp, bias=0.0, scale=-1.0)
            g = scr.tile([P, F], FP32, tag="g")
            nc.scalar.activation(g[:], f[:], Act.Copy, bias=float(EPS), scale=1.0)
            alpha = scr.tile([P, F], FP32, tag="alpha")
            nc.scalar.activation(alpha[:], f[:], Act.Copy, bias=1.0, scale=-1.0)

            gsh = scr.tile([P, F], FP32, tag="gsh")
            nc.scalar.copy(gsh[:, 1:F], g[:, 0:F - 1])
            gsh_seg = gsh[:].rearrange("p (s k) -> p s k", s=segs)
            nc.gpsimd.memset(gsh_seg[:, :, 0:1], 0.0)
            trans = main.tile([P, F], FP32, tag="trans")
            nc.vector.tensor_tensor_scan(trans[:], gsh[:], reset[:], 1.0,
                                         Alu.mult, Alu.max)

            w = main.tile([P, F], FP32, tag="w")
            nc.vector.tensor_tensor(w[:], alpha[:], trans[:], Alu.mult)
            nc.sync.dma_start(wv[b], w[:])

            t_seg = trans[:].rearrange("p (s k) -> p s k", s=segs)
            g_seg = g[:].rearrange("p (s k) -> p s k", s=segs)
            nohit = scr.tile([P, segs], FP32, tag="nohit")
            nc.vector.tensor_tensor(
                nohit[:].unsqueeze(2),
                t_seg[:, :, S - 1:S], g_seg[:, :, S - 1:S], Alu.mult)

            depth_t = scr.tile([P, segs], FP32, tag="depth_t")
            prod = scr.tile([P, F], FP32, tag="prod")
            pscr = scr.tile([P, F], FP32, tag="pscr")
            nc.vector.tensor_tensor(prod[:], w[:], zsrt[:], Alu.mult)
            prod_seg = prod[:].rearrange("p (s k) -> p s k", s=segs)
            pscr_seg = pscr[:].rearrange("p (s k) -> p s k", s=segs)
            for s in range(segs):
                nc.scalar.activation(pscr_seg[:, s], prod_seg[:, s], Act.Copy,
                                     accum_out=depth_t[:, s:s + 1])
            nc.sync.dma_start(depv[b], depth_t[:])

            worig = main.tile([P, F], FP32, tag="worig")
            w_u = w[:].bitcast(U16).rearrange("p (f two) -> p f two", two=2)
            nc.scalar.copy(l16[:], w_u[:, :, 0:1].squeeze(2))
            nc.scalar.copy(h16[:], w_u[:, :, 1:2].squeeze(2))
            nc.gpsimd.local_scatter(hs16[:], h16[:], permg[:],
                                    channels=P, num_elems=F, num_idxs=F)
            nc.gpsimd.local_scatter(ls16[:], l16[:], permg[:],
                                    channels=P, num_elems=F, num_idxs=F)
            wo_u = worig[:].bitcast(U16).rearrange("p (f two) -> p f two", two=2)
            nc.scalar.copy(wo_u[:, :, 0:1].squeeze(2), ls16[:])
            nc.scalar.copy(wo_u[:, :, 1:2].squeeze(2), hs16[:])

            rgbt = main.tile([P, F * 3], FP32, tag="rgbt")
            nc.sync.dma_start(rgbt[:], rgbv[b])
            inst = main.tile([P, F * 3], FP32, tag="inst")
            nc.sync.dma_start(inst[:], insv[b])
            bgt = scr.tile([P, segs * 3], FP32, tag="bgt")
            nc.sync.dma_start(bgt[:], bgv[b])

            color_t = scr.tile([P, segs * 3], FP32, tag="color_t")
            inst_t = scr.tile([P, segs * 3], FP32, tag="inst_t")
            rgb_c = rgbt[:].rearrange("p (f c) -> p f c", c=3)
            ins_c = inst[:].rearrange("p (f c) -> p f c", c=3)
            col_c = color_t[:].rearrange("p (s c) -> p s c", s=segs)
            ino_c = inst_t[:].rearrange("p (s c) -> p s c", s=segs)
            prod2 = scr.tile([P, F], FP32, tag="prod2")
            prod2_seg = prod2[:].rearrange("p (s k) -> p s k", s=segs)
            for c in range(3):
                nc.vector.tensor_tensor(prod[:], worig[:], rgb_c[:, :, c],
                                        Alu.mult)
                for s in range(segs):
                    nc.scalar.activation(pscr_seg[:, s], prod_seg[:, s],
                                         Act.Copy,
                                         accum_out=col_c[:, s:s + 1, c])
                nc.vector.tensor_tensor(prod2[:], worig[:], ins_c[:, :, c],
                                        Alu.mult)
                for s in range(segs):
                    nc.scalar.activation(pscr_seg[:, s], prod2_seg[:, s],
                                         Act.Copy,
                                         accum_out=ino_c[:, s:s + 1, c])
            bgw = scr.tile([P, segs * 3], FP32, tag="bgw")
            bg_seg = bgt[:].rearrange("p (s c) -> p s c", s=segs)
            bgw_seg = bgw[:].rearrange("p (s c) -> p s c", s=segs)
            for s in range(segs):
                nc.vector.tensor_scalar(bgw_seg[:, s], bg_seg[:, s],
                                        nohit[:, s:s + 1], None, Alu.mult)
            nc.vector.tensor_tensor(color_t[:], color_t[:], bgw[:], Alu.add)
            nc.sync.dma_start(colv[b], color_t[:])
            nc.sync.dma_start(iov[b], inst_t[:])

    nc.compile()
    return nc


def _get_nc():
    key = (N_RAYS // N_CORES,)
    if key not in _cache:
        _cache[key] = _build(N_RAYS // N_CORES)
    return _cache[key]


def _shard(inputs):
    z = np.ascontiguousarray(np.asarray(inputs["z_vals"], dtype=np.float32))
    sg = np.ascontiguousarray(np.asarray(inputs["sigma_vals"], dtype=np.float32))
    rgb = np.asarray(inputs["rgb_vals"], dtype=np.float32)
    ins = np.asarray(inputs["instance_vals"], dtype=np.float32)
    bg = np.ascontiguousarray(np.asarray(inputs["bg_color"], dtype=np.float32))
    n, s = z.shape
    per = n // N_CORES
    rgb2 = np.ascontiguousarray(rgb.reshape(n, s * 3))
    ins2 = np.ascontiguousarray(ins.reshape(n, s * 3))
    maps = []
    for c in range(N_CORES):
        sl = slice(c * per, (c + 1) * per)
        maps.append({"z": z[sl], "sigma": sg[sl], "rgb": rgb2[sl],
                     "inst": ins2[sl], "bg": bg[sl]})
    return maps


def run_sharded(inputs, trace=False):
    from concourse import bass_utils
    nc = _get_nc()
    maps = _shard(inputs)
    res = bass_utils.run_bass_kernel_spmd(
        nc, maps, core_ids=list(range(N_CORES)), trace=trace)
    color = np.concatenate([r["color"] for r in res.results], 0)
    depth = np.concatenate([r["depth"] for r in res.results], 0)[:, 0]
    instance = np.concatenate([r["instance"] for r in res.results], 0)
    w = np.concatenate([r["w"] for r in res.results], 0)
    return (color, depth, instance, w), res


def kernel(**inputs):
    outs, _ = run_sharded(inputs, trace=False)
    return outs
